# revision 10
# baseline (speedup 1.0000x reference)
"""BigGNN Trainium2 kernel: 2 TransformerConv graphs + dense cross-attention + MLP head.

Sharding: 8 cores; core c -> graph g=c//4, dst-row range [96r, 96r+96), r=c%4.
Phase A: sparse TransformerConv per graph, dst-sharded (no cross-core reduction).
AllGather (bf16) -> Phase B: dense cross-attention (query rows local).
AllGather of pooled partials (tiny) -> MLP head, replicated.

Compute dtype: bf16 matmul operands, f32 PSUM accumulation and softmax math.
Biases live in padding row 300/600 of the weight tensors (harmless in normal
matmuls since x padding rows are zero; applied explicitly via K=1 matmuls).
"""

import os
import numpy as np
import ml_dtypes

import concourse.bacc as bacc
import concourse.mybir as mybir
import concourse.tile as tile
from concourse.bass_utils import run_bass_kernel_spmd

F32 = mybir.dt.float32
BF16 = mybir.dt.bfloat16
I16 = mybir.dt.int16
U16 = mybir.dt.uint16
AL = mybir.AluOpType
AF = mybir.ActivationFunctionType
BF = ml_dtypes.bfloat16

NCORES = 8
NB = 384          # nodes per graph
CF = 300          # feature dim
CFP = 384         # padded feature (K) dim
HC = 600          # 2 heads * 300
RPC = 96          # dst rows per core
PADW = 384        # allgather row width in bf16 (768B, %256==0)
KVW = 1280        # packed k|v row width in bf16
ISQ = float(1.0 / np.sqrt(300.0))
EPS = 1e-16

_PROG_CACHE = {}
LAST_EXEC_NS = None
KSTAGE = int(os.environ.get("KSTAGE", "99"))


def _build(E_pad, stage=99):
    T = E_pad // 128
    nc = bacc.Bacc("TRN2", target_bir_lowering=False, debug=False, num_devices=NCORES)

    def din(name, shape, dt=BF16):
        return nc.dram_tensor(name, shape, dt, kind="ExternalInput")

    xT = din("xT", [CFP, NB])
    xT96 = din("xT96", [CFP, RPC])
    eaT = din("eaT", [CFP, E_pad])
    dstl = din("dstl", [128, T], F32)
    srcidx = din("srcidx", [128, E_pad // 16], I16)
    xoidx = din("xoidx", [128, NB // 16], I16)
    Wa = din("Wa", [CFP, 2400])          # q|k|v|We cols; row 320 = bq|bk|bv|0
    Wska = din("Wska", [CFP, CF])        # row 320 = bskip
    Wb = din("Wb", [CFP, 2400])          # row 320 = bq_b|bk_b|bv_b|0
    Wskb = din("Wskb", [CFP, CF])        # row 320 = bskip_b
    bcolB = din("bcolB", [128, 6, 2], F32)  # [:,3h+ci,0]=bk_b chunk, [:,.,1]=bq_b chunk
    W1 = din("W1", [768, 600])           # row 640 = b1
    W2 = din("W2", [768, 300])           # row 640 = b2
    W3 = din("W3", [CFP, 1])             # row 320 = b3
    cbf = din("cbf", [128, 224])         # identb(0:128)|iota96(128:224)
    cones = din("cones", [128, 128])     # all-ones (rows sliced at any base partition)
    selmat = din("selmat", [8, 2])       # phase-C pool selector

    y = nc.dram_tensor("y", [1, 608], F32, kind="ExternalOutput")

    CW = [128, 128, 44]   # within-head 300-chunking
    with tile.TileContext(nc) as tc:
        with (
            tc.tile_pool(name="cpool", bufs=1) as cp,
            tc.tile_pool(name="dram", bufs=1, space="DRAM") as dp,
            tc.tile_pool(name="accp", bufs=1, space="PSUM") as apx,
        ):
            # ---- constants + phase-B/C weights (loaded up-front, overlap phase A) ----
            cbf_sb = cp.tile([128, 224], BF16)
            cones_sb = cp.tile([128, 128], BF16)
            sel_sb = cp.tile([8, 2], BF16)
            xo_sb = cp.tile([128, NB // 16], I16)
            nc.sync.dma_start(cbf_sb[:], cbf[:])
            nc.sync.dma_start(cones_sb[:], cones[:])
            nc.sync.dma_start(sel_sb[:], selmat[:])
            nc.sync.dma_start(xo_sb[:], xoidx[:])
            identb = cbf_sb[:, 0:128]
            iota_sb = cbf_sb[:, 128:224]

            W1_t = [cp.tile([128, 600], BF16, tag=f"W1_{k}", name=f"W1_{k}") for k in range(6)]
            W2_t = [cp.tile([128, 300], BF16, tag=f"W2_{k}", name=f"W2_{k}") for k in range(6)]
            W3_t = [cp.tile([128, 1], BF16, tag=f"W3_{k}", name=f"W3_{k}") for k in range(3)]
            for k in range(6):
                nc.sync.dma_start(W1_t[k][:], W1[128 * k : 128 * k + 128, :])
                nc.sync.dma_start(W2_t[k][:], W2[128 * k : 128 * k + 128, :])
            for k in range(3):
                nc.sync.dma_start(W3_t[k][:], W3[128 * k : 128 * k + 128, :])
            Wb_t = [cp.tile([128, 2400], BF16, tag=f"Wb_{k}", name=f"Wb_{k}") for k in range(3)]
            Wskb_t = [cp.tile([128, CF], BF16, tag=f"Wskb_{k}", name=f"Wskb_{k}") for k in range(3)]
            bcol_sb = cp.tile([128, 6, 2], F32)
            for k in range(3):
                nc.sync.dma_start(Wb_t[k][:], Wb[128 * k : 128 * k + 128, :])
                nc.sync.dma_start(Wskb_t[k][:], Wskb[128 * k : 128 * k + 128, :])
            nc.sync.dma_start(bcol_sb[:], bcolB[:])

            ag1_in = dp.tile([RPC, PADW], BF16)
            ag1_out = dp.tile([NCORES * RPC, PADW], BF16)
            ag2_in = dp.tile([1, PADW], BF16)
            ag2_out = dp.tile([NCORES, PADW], BF16)
            kv_dram = dp.tile([NB, KVW], BF16)
            h_dram = dp.tile([1, 600], F32)

            # persistent PSUM accumulators (scatter)
            accA = apx.tile([RPC, 302], F32, tag="accA")
            accB = apx.tile([RPC, 302], F32, tag="accB")

            with tc.tile_pool(name="bpool", bufs=1) as bp:
                final96 = bp.tile([RPC, PADW], BF16)
                myT = [bp.tile([128, RPC], BF16, tag=f"myT_{k}", name=f"myT_{k}") for k in range(3)]

                # ================= PHASE A =================
                with (
                    tc.tile_pool(name="apool", bufs=1) as ap,
                    tc.tile_pool(name="awork", bufs=3) as aw,
                ):
                    xT_t = [ap.tile([128, NB], BF16, tag=f"xT_{k}", name=f"xT_{k}") for k in range(3)]
                    xT96_t = [ap.tile([128, RPC], BF16, tag=f"xT96_{k}", name=f"xT96_{k}") for k in range(3)]
                    Wa_t = [ap.tile([128, 2400], BF16, tag=f"Wa_{k}", name=f"Wa_{k}") for k in range(3)]
                    Wska_t = [ap.tile([128, CF], BF16, tag=f"Wska_{k}", name=f"Wska_{k}") for k in range(3)]
                    eaT_t = [ap.tile([128, E_pad], BF16, tag=f"eaT_{k}", name=f"eaT_{k}") for k in range(3)]
                    dstl_sb = ap.tile([128, T], F32)
                    src_sb = ap.tile([128, E_pad // 16], I16)
                    for k in range(3):
                        nc.sync.dma_start(xT_t[k][:], xT[128 * k : 128 * k + 128, :])
                        nc.sync.dma_start(xT96_t[k][:], xT96[128 * k : 128 * k + 128, :])
                        nc.sync.dma_start(Wa_t[k][:], Wa[128 * k : 128 * k + 128, :])
                        nc.sync.dma_start(Wska_t[k][:], Wska[128 * k : 128 * k + 128, :])
                        nc.sync.dma_start(eaT_t[k][:], eaT[128 * k : 128 * k + 128, :])
                    nc.sync.dma_start(dstl_sb[:], dstl[:])
                    nc.sync.dma_start(src_sb[:], srcidx[:])

                    S_all = ap.tile([128, T * RPC], BF16)
                    ST_all = ap.tile([RPC, T * 128], BF16)
                    q96_sb = ap.tile([RPC, HC], BF16)
                    skip_sb = ap.tile([RPC, CF], F32)
                    kv_sb = [ap.tile([128, KVW], BF16, tag=f"kvsb_{m}", name=f"kvsb_{m}") for m in range(3)]

                    ba = Wa_t[2][64:65, :]      # bias row 320
                    bska = Wska_t[2][64:65, :]

                    # --- projections + S build ---
                    with tc.tile_pool(name="pre_ps", bufs=2, space="PSUM") as pp:
                        for m in range(3):
                            for h in range(2):
                                for off, dst0 in ((600, 0), (1200, 640)):  # k, v
                                    ps = pp.tile([128, CF], F32, tag="pj")
                                    for kc in range(3):
                                        nc.tensor.matmul(
                                            ps[:],
                                            xT_t[kc][:, 128 * m : 128 * m + 128],
                                            Wa_t[kc][:, off + 300 * h : off + 300 * h + 300],
                                            start=(kc == 0), stop=False,
                                        )
                                    nc.tensor.matmul(
                                        ps[:], cones_sb[64:65, 0:128],
                                        ba[:, off + 300 * h : off + 300 * h + 300],
                                        start=False, stop=True,
                                    )
                                    nc.scalar.copy(kv_sb[m][:, dst0 + 300 * h : dst0 + 300 * h + 300], ps[:])
                        for h in range(2):
                            ps = pp.tile([RPC, CF], F32, tag="pj")
                            for kc in range(3):
                                nc.tensor.matmul(
                                    ps[:], xT96_t[kc][:],
                                    Wa_t[kc][:, 300 * h : 300 * h + 300],
                                    start=(kc == 0), stop=False,
                                )
                            nc.tensor.matmul(
                                ps[:], cones_sb[64:65, 0:RPC], ba[:, 300 * h : 300 * h + 300],
                                start=False, stop=True,
                            )
                            nc.scalar.copy(q96_sb[:, 300 * h : 300 * h + 300], ps[:])
                        ps = pp.tile([RPC, CF], F32, tag="pj")
                        for kc in range(3):
                            nc.tensor.matmul(ps[:], xT96_t[kc][:], Wska_t[kc][:],
                                             start=(kc == 0), stop=False)
                        nc.tensor.matmul(ps[:], cones_sb[64:65, 0:RPC], bska[:],
                                         start=False, stop=True)
                        nc.scalar.copy(skip_sb[:], ps[:])
                        for t in range(T):
                            nc.vector.tensor_scalar(
                                S_all[:, RPC * t : RPC * t + RPC], iota_sb,
                                dstl_sb[:, t : t + 1], None, AL.is_equal,
                            )
                            st_ps = pp.tile([RPC, 128], BF16, tag="st")
                            nc.tensor.transpose(st_ps[:], S_all[:, RPC * t : RPC * t + RPC], identb)
                            nc.scalar.copy(ST_all[:, 128 * t : 128 * t + 128], st_ps[:])

                    for m in range(3):
                        nc.vector.memset(kv_sb[m][:, 600:640], 0.0)
                        nc.vector.memset(kv_sb[m][:, 1240:1280], 0.0)
                        nc.sync.dma_start(kv_dram[128 * m : 128 * m + 128, :], kv_sb[m][:])
                    kvg = ap.tile([128, T, KVW], BF16)
                    if stage < 2:
                        nc.vector.memset(kvg[:].bitcast(U16), 0)
                    if stage >= 2:
                        nc.gpsimd.dma_gather(
                            out_ap=kvg[:], in_ap=kv_dram[:], idxs_ap=src_sb[:],
                            num_idxs=E_pad, num_idxs_reg=E_pad, elem_size=KVW,
                            single_packet=False,
                        )

                    # --- edge loop ---
                    if stage >= 3:
                        with tc.tile_pool(name="edge_ps", bufs=1, space="PSUM") as ep:
                            for t in range(T):
                                e_ps = [ep.tile([128, CF], F32, tag=f"eps{h}", name=f"eps{h}", bufs=2) for h in range(2)]
                                for h in range(2):
                                    for kc in range(3):
                                        nc.tensor.matmul(
                                            e_ps[h][:],
                                            eaT_t[kc][:, 128 * t : 128 * t + 128],
                                            Wa_t[kc][:, 1800 + 300 * h : 1800 + 300 * h + 300],
                                            start=(kc == 0), stop=(kc == 2),
                                        )
                                qg_ps = [ep.tile([128, CF], F32, tag=f"qg{h}", name=f"qg{h}", bufs=1) for h in range(2)]
                                for h in range(2):
                                    nc.tensor.matmul(
                                        qg_ps[h][:], ST_all[:, 128 * t : 128 * t + 128],
                                        q96_sb[:, 300 * h : 300 * h + 300],
                                        start=True, stop=True,
                                    )
                                e_sb = aw.tile([128, HC], BF16, tag="e_sb")
                                for h in range(2):
                                    nc.scalar.copy(e_sb[:, 300 * h : 300 * h + 300], e_ps[h][:])
                                kj_sb = aw.tile([128, HC], BF16, tag="kj_sb")
                                nc.vector.tensor_tensor(kj_sb[:], kvg[:, t, 0:600], e_sb[:], op=AL.add)
                                prod_sb = aw.tile([128, HC], F32, tag="prod_sb")
                                for h in range(2):
                                    nc.vector.tensor_tensor(
                                        prod_sb[:, 300 * h : 300 * h + 300], qg_ps[h][:],
                                        kj_sb[:, 300 * h : 300 * h + 300], op=AL.mult,
                                    )
                                alpha_sb = aw.tile([128, 2], F32, tag="alpha_sb")
                                nc.vector.tensor_reduce(
                                    alpha_sb[:], prod_sb[:].rearrange("p (h c) -> p h c", h=2),
                                    axis=mybir.AxisListType.X, op=AL.add,
                                )
                                w_sb = aw.tile([128, 2], F32, tag="w_sb")
                                nc.scalar.activation(w_sb[:], alpha_sb[:], AF.Exp, scale=ISQ)
                                vj_sb = aw.tile([128, HC], BF16, tag="vj_sb")
                                nc.vector.tensor_tensor(vj_sb[:], kvg[:, t, 640:1240], e_sb[:], op=AL.add)
                                pay_sb = aw.tile([128, 604], BF16, tag="pay_sb")
                                for h in range(2):
                                    nc.vector.tensor_scalar(
                                        pay_sb[:, 302 * h : 302 * h + 300],
                                        vj_sb[:, 300 * h : 300 * h + 300],
                                        w_sb[:, h : h + 1], None, AL.mult,
                                    )
                                    nc.scalar.copy(pay_sb[:, 302 * h + 300 : 302 * h + 301], w_sb[:, h : h + 1])
                                    nc.vector.memset(pay_sb[:, 302 * h + 301 : 302 * h + 302], 0.0)
                                nc.tensor.matmul(accA[:], S_all[:, RPC * t : RPC * t + RPC],
                                                 pay_sb[:, 0:302], start=(t == 0), stop=(t == T - 1))
                                nc.tensor.matmul(accB[:], S_all[:, RPC * t : RPC * t + RPC],
                                                 pay_sb[:, 302:604], start=(t == 0), stop=(t == T - 1))

                    # --- finalize phase A ---
                    if stage < 3:
                        nc.vector.memset(final96[:].bitcast(U16), 0)
                    if stage >= 3:
                        den_sb = aw.tile([RPC, 2], F32, tag="den_sb")
                        nc.scalar.copy(den_sb[:, 0:1], accA[:, 300:301])
                        nc.scalar.copy(den_sb[:, 1:2], accB[:, 300:301])
                        nc.vector.tensor_scalar(den_sb[:], den_sb[:], EPS, None, AL.add)
                        dinv_sb = aw.tile([RPC, 2], F32, tag="dinv_sb")
                        nc.vector.reciprocal(dinv_sb[:], den_sb[:])
                        o_sb = aw.tile([RPC, HC], F32, tag="o_sb")
                        nc.vector.tensor_scalar(o_sb[:, 0:300], accA[:, 0:300], dinv_sb[:, 0:1], None, AL.mult)
                        nc.vector.tensor_scalar(o_sb[:, 300:600], accB[:, 0:300], dinv_sb[:, 1:2], None, AL.mult)
                        m_sb = aw.tile([RPC, CF], F32, tag="m_sb")
                        nc.vector.tensor_tensor(m_sb[:], o_sb[:, 0:300], o_sb[:, 300:600], op=AL.add)
                        nc.vector.tensor_scalar(m_sb[:], m_sb[:], 0.5, None, AL.mult)
                        nc.vector.tensor_tensor(m_sb[:], m_sb[:], skip_sb[:], op=AL.add)
                        lk = aw.tile([RPC, CF], F32, tag="lk")
                        nc.scalar.mul(lk[:], m_sb[:], 0.01)
                        nc.vector.memset(final96[:].bitcast(U16), 0)
                        nc.vector.tensor_tensor(final96[:, 0:300], m_sb[:], lk[:], op=AL.max)

                if stage >= 4:
                    nc.gpsimd.dma_start(ag1_in[:], final96[:])
                    nc.gpsimd.collective_compute(
                        "AllGather", AL.bypass, ins=[ag1_in.opt()], outs=[ag1_out.opt()],
                        replica_groups=[list(range(NCORES))],
                    )

                # ================= PHASE B =================
                if stage >= 5:
                    with (
                        tc.tile_pool(name="bwork", bufs=1) as bw,
                        tc.tile_pool(name="b_ps", bufs=1, space="PSUM") as bps,
                    ):
                        XO = bw.tile([128, 3, PADW], BF16)
                        nc.gpsimd.dma_gather(
                            out_ap=XO[:], in_ap=ag1_out[:], idxs_ap=xo_sb[:],
                            num_idxs=NB, num_idxs_reg=NB, elem_size=PADW,
                        )
                        for j in range(3):
                            w = CW[j]
                            tp = bps.tile([128, 128], BF16, tag="tr", bufs=2, name="tp")
                            nc.tensor.transpose(tp[0:w, 0:RPC], final96[:, 128 * j : 128 * j + w],
                                                identb[0:RPC, 0:RPC])
                            if w < 128:
                                nc.vector.memset(myT[j][:].bitcast(U16), 0)
                            nc.scalar.copy(myT[j][0:w, :], tp[0:w, 0:RPC])
                        XOT_t = [bw.tile([128, NB], BF16, tag=f"XOT_{k}", name=f"XOT_{k}") for k in range(3)]
                        for j in range(3):
                            w = CW[j]
                            if w < 128:
                                nc.vector.memset(XOT_t[j][:].bitcast(U16), 0)
                            for i in range(3):
                                tp = bps.tile([128, 128], BF16, tag="tr", bufs=2, name="tp")
                                nc.tensor.transpose(tp[0:w, 0:128], XO[:, i, 128 * j : 128 * j + w],
                                                    identb)
                                nc.scalar.copy(XOT_t[j][0:w, 128 * i : 128 * i + 128], tp[0:w, 0:128])

                        bb = Wb_t[2][64:65, :]
                        bskb = Wskb_t[2][64:65, :]
                        kT_sb = [bw.tile([128, NB], BF16, tag=f"kT_{i}", name=f"kT_{i}", bufs=1) for i in range(6)]
                        qT_sb = [bw.tile([128, RPC], BF16, tag=f"qT_{i}", name=f"qT_{i}", bufs=1) for i in range(6)]
                        for h in range(2):
                            for ci in range(3):
                                w = CW[ci]
                                off = 300 * h + 128 * ci
                                e0 = bps.tile([128, 2], F32, tag="pb", bufs=4, name="e0")
                                for kc in range(3):
                                    nc.tensor.matmul(e0[0:w, :], Wb_t[kc][:, 1800 + off : 1800 + off + w],
                                                     cones_sb[:, 0:2], start=(kc == 0), stop=(kc == 2))
                                kb = bw.tile([128, 1], F32, tag="kb", bufs=2, name="kb")
                                nc.vector.tensor_tensor(kb[0:w, :], e0[0:w, 0:1],
                                                        bcol_sb[0:w, 3 * h + ci, 0:1], op=AL.add)
                                ps = bps.tile([128, NB], F32, tag="pb", bufs=4, name="psk")
                                for kc in range(3):
                                    nc.tensor.matmul(ps[0:w, :], Wb_t[kc][:, 600 + off : 600 + off + w],
                                                     XOT_t[kc][:], start=(kc == 0), stop=(kc == 2))
                                nc.vector.tensor_scalar(kT_sb[3 * h + ci][0:w, :], ps[0:w, :],
                                                        kb[0:w, 0:1], None, AL.add)
                                ps2 = bps.tile([128, RPC], F32, tag="pb", bufs=4, name="psq")
                                for kc in range(3):
                                    nc.tensor.matmul(ps2[0:w, :], Wb_t[kc][:, off : off + w],
                                                     myT[kc][:], start=(kc == 0), stop=(kc == 2))
                                nc.vector.tensor_scalar(qT_sb[3 * h + ci][0:w, :], ps2[0:w, :],
                                                        bcol_sb[0:w, 3 * h + ci, 1:2], None, AL.add)

                        w_f = [bw.tile([RPC, NB], BF16, tag=f"wf_{h}", name=f"wf_{h}", bufs=1) for h in range(2)]
                        dinvB = bw.tile([RPC, 2], F32, tag="dinvB", bufs=1)
                        denB = bw.tile([RPC, 2], F32, tag="denB", bufs=1)
                        for h in range(2):
                            al = bps.tile([RPC, NB], F32, tag="pb", bufs=4, name="al")
                            for ci in range(3):
                                w = CW[ci]
                                nc.tensor.matmul(al[:], qT_sb[3 * h + ci][0:w, :], kT_sb[3 * h + ci][0:w, :],
                                                 start=(ci == 0), stop=(ci == 2))
                            nc.scalar.activation(w_f[h][:], al[:], AF.Exp, scale=ISQ)
                            nc.vector.tensor_reduce(denB[:, h : h + 1], w_f[h][:],
                                                    axis=mybir.AxisListType.X, op=AL.add)
                        nc.vector.tensor_scalar(denB[:], denB[:], EPS, None, AL.add)
                        nc.vector.reciprocal(dinvB[:], denB[:])
                        wT_sb = [bw.tile([128, RPC], BF16, tag=f"wT_{i}", name=f"wT_{i}", bufs=1) for i in range(6)]
                        for h in range(2):
                            for m in range(3):
                                tp = bps.tile([128, 128], BF16, tag="tr", bufs=2, name="tp")
                                nc.tensor.transpose(tp[:, 0:RPC], w_f[h][:, 128 * m : 128 * m + 128],
                                                    identb[0:RPC, 0:RPC])
                                nc.scalar.copy(wT_sb[3 * h + m][:], tp[:, 0:RPC])

                        # vbias row = bv_b + e0_row
                        vb_ps = [bps.tile([1, CF], F32, tag="pb", bufs=4, name="vb") for _ in range(2)]
                        for h in range(2):
                            for kc in range(3):
                                nc.tensor.matmul(vb_ps[h][:], cones_sb[:, 0:1],
                                                 Wb_t[kc][:, 1800 + 300 * h : 1800 + 300 * h + 300],
                                                 start=(kc == 0), stop=False)
                            nc.tensor.matmul(vb_ps[h][:], cones_sb[64:65, 0:1],
                                             bb[:, 1200 + 300 * h : 1200 + 300 * h + 300],
                                             start=False, stop=True)
                        vbias_sb = bw.tile([1, HC], BF16, tag="vbias", bufs=1)
                        for h in range(2):
                            nc.scalar.copy(vbias_sb[:, 300 * h : 300 * h + 300], vb_ps[h][:])

                        vv_sb = [bw.tile([128, HC], BF16, tag=f"vv_{m}", name=f"vv_{m}", bufs=1) for m in range(3)]
                        for m in range(3):
                            for h in range(2):
                                ps = bps.tile([128, CF], F32, tag="pb", bufs=4, name="psv")
                                for kc in range(3):
                                    nc.tensor.matmul(ps[:], XOT_t[kc][:, 128 * m : 128 * m + 128],
                                                     Wb_t[kc][:, 1200 + 300 * h : 1200 + 300 * h + 300],
                                                     start=(kc == 0), stop=False)
                                nc.tensor.matmul(ps[:], cones_sb[0:1, 0:128],
                                                 vbias_sb[:, 300 * h : 300 * h + 300],
                                                 start=False, stop=True)
                                nc.scalar.copy(vv_sb[m][:, 300 * h : 300 * h + 300], ps[:])

                        obo = bw.tile([RPC, HC], F32, tag="obo", bufs=1)
                        for h in range(2):
                            av = bps.tile([RPC, CF], F32, tag="pb", bufs=4, name="av")
                            for m in range(3):
                                nc.tensor.matmul(av[:], wT_sb[3 * h + m][:],
                                                 vv_sb[m][:, 300 * h : 300 * h + 300],
                                                 start=(m == 0), stop=(m == 2))
                            nc.vector.tensor_scalar(obo[:, 300 * h : 300 * h + 300], av[:],
                                                    dinvB[:, h : h + 1], None, AL.mult)
                        skB = bps.tile([RPC, CF], F32, tag="pb", bufs=4, name="skB")
                        for kc in range(3):
                            nc.tensor.matmul(skB[:], myT[kc][:], Wskb_t[kc][:],
                                             start=(kc == 0), stop=False)
                        nc.tensor.matmul(skB[:], cones_sb[64:65, 0:RPC], bskb[:],
                                         start=False, stop=True)
                        mB = bw.tile([RPC, CF], F32, tag="mB", bufs=1)
                        nc.vector.tensor_tensor(mB[:], obo[:, 0:300], obo[:, 300:600], op=AL.add)
                        nc.vector.tensor_scalar(mB[:], mB[:], 0.5, None, AL.mult)
                        nc.vector.tensor_tensor(mB[:], mB[:], skB[:], op=AL.add)
                        lkB = bw.tile([RPC, CF], F32, tag="lkB", bufs=1)
                        nc.scalar.mul(lkB[:], mB[:], 0.01)
                        finalB = bw.tile([RPC, PADW], BF16, tag="finalB", bufs=1)
                        nc.vector.memset(finalB[:].bitcast(U16), 0)
                        nc.vector.tensor_tensor(finalB[:, 0:300], mB[:], lkB[:], op=AL.max)
                        # pooled partial (column sums of my 96 rows)
                        pool_ps = bps.tile([1, PADW], F32, tag="pb", bufs=4, name="pool_ps")
                        nc.tensor.matmul(pool_ps[:], cones_sb[0:RPC, 0:1], finalB[:],
                                         start=True, stop=True)
                        pool_sb = bw.tile([1, PADW], BF16, tag="pool_sb", bufs=1)
                        nc.scalar.copy(pool_sb[:], pool_ps[:])
                        nc.gpsimd.dma_start(ag2_in[:], pool_sb[:])

                if stage >= 7:
                    nc.gpsimd.collective_compute(
                        "AllGather", AL.bypass, ins=[ag2_in.opt()], outs=[ag2_out.opt()],
                        replica_groups=[list(range(NCORES))],
                    )

            # ================= PHASE C =================
            if stage < 8:
                with tc.tile_pool(name="fbpool", bufs=1) as fbp:
                    y_sb0 = fbp.tile([1, 608], F32)
                    nc.vector.memset(y_sb0[:], 0.0)
                    nc.sync.dma_start(y[:], y_sb0[:])
            if stage >= 8:
                with (
                    tc.tile_pool(name="cwork", bufs=1) as cw,
                    tc.tile_pool(name="c_ps", bufs=1, space="PSUM") as cps,
                ):
                    XF2 = cw.tile([8, PADW], BF16)
                    nc.sync.dma_start(XF2[:], ag2_out[:])
                    pool2 = cps.tile([2, PADW], F32, tag="mc", bufs=4)
                    nc.tensor.matmul(pool2[:], sel_sb[:], XF2[:], start=True, stop=True)
                    pools_sb = cw.tile([2, PADW], F32)
                    nc.scalar.mul(pools_sb[:], pool2[:], 1.0 / NB)
                    nc.sync.dma_start(h_dram[0:1, 0:300], pools_sb[0:1, 0:300])
                    nc.sync.dma_start(h_dram[0:1, 300:600], pools_sb[1:2, 0:300])
                    y_sb = cw.tile([1, 608], F32)
                    nc.vector.memset(y_sb[:], 0.0)
                    nc.sync.dma_start(y_sb[:, 0:600], h_dram[:])
                    hT_t = [cw.tile([128, 2], BF16, tag=f"hT_{k}", name=f"hT_{k}") for k in range(5)]
                    for k in range(5):
                        mw = min(128, 600 - 128 * k)
                        nc.vector.memset(hT_t[k][:].bitcast(U16), 0)
                        nc.gpsimd.dma_start(hT_t[k][0:mw, 0:1], h_dram[0:1, 128 * k : 128 * k + mw])
                    # MLP layer 1 (bias via K=1 matmul from W1 row 600)
                    h1T_t = [cw.tile([128, 2], BF16, tag=f"h1T_{k}", name=f"h1T_{k}") for k in range(5)]
                    for mi in range(5):
                        mw = min(128, 600 - 128 * mi)
                        ps = cps.tile([128, 2], F32, tag="mc", bufs=4, name="ps1")
                        for kc in range(5):
                            nc.tensor.matmul(ps[0:mw, :], W1_t[kc][:, 128 * mi : 128 * mi + mw],
                                             hT_t[kc][:], start=(kc == 0), stop=False)
                        nc.tensor.matmul(ps[0:mw, :], W1_t[5][0:1, 128 * mi : 128 * mi + mw],
                                         cones_sb[0:1, 0:2], start=False, stop=True)
                        lkc = cw.tile([128, 2], F32, tag="lkc", bufs=2, name="lkc")
                        nc.scalar.mul(lkc[0:mw, :], ps[0:mw, :], 0.01)
                        nc.vector.memset(h1T_t[mi][:].bitcast(U16), 0)
                        nc.vector.tensor_tensor(h1T_t[mi][0:mw, :], ps[0:mw, :], lkc[0:mw, :], op=AL.max)
                    # MLP layer 2 (bias from W2 row 600)
                    h2T_t = [cw.tile([128, 2], BF16, tag=f"h2T_{k}", name=f"h2T_{k}") for k in range(3)]
                    for mi in range(3):
                        mw = min(128, 300 - 128 * mi)
                        ps = cps.tile([128, 2], F32, tag="mc", bufs=4, name="ps2")
                        for kc in range(5):
                            nc.tensor.matmul(ps[0:mw, :], W2_t[kc][:, 128 * mi : 128 * mi + mw],
                                             h1T_t[kc][:], start=(kc == 0), stop=False)
                        nc.tensor.matmul(ps[0:mw, :], W2_t[5][0:1, 128 * mi : 128 * mi + mw],
                                         cones_sb[0:1, 0:2], start=False, stop=True)
                        lkc = cw.tile([128, 2], F32, tag="lkc", bufs=2, name="lkc")
                        nc.scalar.mul(lkc[0:mw, :], ps[0:mw, :], 0.01)
                        nc.vector.memset(h2T_t[mi][:].bitcast(U16), 0)
                        nc.vector.tensor_tensor(h2T_t[mi][0:mw, :], ps[0:mw, :], lkc[0:mw, :], op=AL.max)
                    # MLP layer 3 + sigmoid (bias from W3 row 300)
                    ps3 = cps.tile([1, 2], F32, tag="mc", bufs=4)
                    for kc in range(3):
                        nc.tensor.matmul(ps3[:], W3_t[kc][:], h2T_t[kc][:],
                                         start=(kc == 0), stop=False)
                    nc.tensor.matmul(ps3[:], W3_t[2][64:65, 0:1], cones_sb[64:65, 0:2],
                                     start=False, stop=True)
                    nc.scalar.activation(y_sb[:, 600:601], ps3[:, 0:1], AF.Sigmoid)
                    nc.sync.dma_start(y[:], y_sb[:])

    nc.compile()
    return nc


def _prep_core_inputs(c, xs, eis, eas, params, buckets, E_pad):
    g, r = c // 4, c % 4
    T = E_pad // 128
    sel = buckets[(g, r)]
    n = len(sel)
    src = np.asarray(eis[g][0])[sel].astype(np.int64)
    dst = np.asarray(eis[g][1])[sel].astype(np.int64)

    pa = params["tsa"][0] if g == 0 else params["gsa"][0]
    pb = params["tca"][0] if g == 0 else params["gca"][0]
    mlp = params["mlp"]
    A = lambda v: np.asarray(v, dtype=np.float32)

    own = A(xs[g])
    xT = np.zeros((CFP, NB), BF)
    xT[:300, :] = own.T
    xT96 = np.zeros((CFP, RPC), BF)
    xT96[:300, :] = own[96 * r : 96 * r + 96].T

    ea = A(eas[g])[sel]
    eaT = np.zeros((CFP, E_pad), BF)
    eaT[:300, :n] = ea.T

    dstl = np.full(E_pad, -1.0, np.float32)
    dstl[:n] = (dst - 96 * r).astype(np.float32)
    dstl = dstl.reshape(T, 128).T.copy()

    s = np.zeros(E_pad, np.int64)
    s[:n] = src
    srcidx = np.tile(s.reshape(-1, 16).T, (8, 1)).astype(np.int16)

    xo = (1 - g) * NB + np.arange(NB)
    xoidx = np.tile(xo.reshape(-1, 16).T, (8, 1)).astype(np.int16)

    def wpack(p):
        W = np.zeros((CFP, 2400), BF)
        W[:300, 0:600] = A(p["Wq"]); W[:300, 600:1200] = A(p["Wk"])
        W[:300, 1200:1800] = A(p["Wv"]); W[:300, 1800:2400] = A(p["We"])
        W[320, 0:600] = A(p["bq"]); W[320, 600:1200] = A(p["bk"])
        W[320, 1200:1800] = A(p["bv"])
        Wsk = np.zeros((CFP, CF), BF)
        Wsk[:300] = A(p["Wskip"]); Wsk[320] = A(p["bskip"])
        return W, Wsk

    Wa, Wska = wpack(pa)
    Wb, Wskb = wpack(pb)

    bcolB = np.zeros((128, 6, 2), np.float32)
    bkb, bqb = A(pb["bk"]), A(pb["bq"])
    for h in range(2):
        for ci in range(3):
            w = [128, 128, 44][ci]
            off = 300 * h + 128 * ci
            bcolB[:w, 3 * h + ci, 0] = bkb[off : off + w]
            bcolB[:w, 3 * h + ci, 1] = bqb[off : off + w]

    W1 = np.zeros((768, 600), BF); W1[:600] = A(mlp["W1"]); W1[640] = A(mlp["b1"])
    W2 = np.zeros((768, 300), BF); W2[:600] = A(mlp["W2"]); W2[640] = A(mlp["b2"])
    W3 = np.zeros((CFP, 1), BF); W3[:300] = A(mlp["W3"]); W3[320, 0] = A(mlp["b3"])[0]

    cbf = np.zeros((128, 224), BF)
    cbf[:, 0:128] = np.eye(128)
    cbf[:, 128:224] = np.arange(RPC)[None, :]

    selmat = np.zeros((8, 2), BF)
    selmat[0:4, 0] = 1.0
    selmat[4:8, 1] = 1.0

    return {
        "xT": xT, "xT96": xT96, "eaT": eaT, "dstl": dstl, "srcidx": srcidx,
        "xoidx": xoidx, "Wa": Wa, "Wska": Wska, "Wb": Wb, "Wskb": Wskb,
        "bcolB": bcolB, "W1": W1, "W2": W2, "W3": W3, "cbf": cbf,
        "cones": np.ones((128, 128), BF), "selmat": selmat,
    }


def kernel(x_1, x_2, edge_idx_1, edge_idx_2, edge_attr_1, edge_attr_2, params):
    global LAST_EXEC_NS
    xs = [np.asarray(x_1, np.float32), np.asarray(x_2, np.float32)]
    eis = [np.asarray(edge_idx_1), np.asarray(edge_idx_2)]
    eas = [np.asarray(edge_attr_1, np.float32), np.asarray(edge_attr_2, np.float32)]

    buckets = {}
    maxn = 0
    for g in range(2):
        dst = eis[g][1].astype(np.int64)
        for r in range(4):
            sel = np.where((dst >= 96 * r) & (dst < 96 * (r + 1)))[0]
            buckets[(g, r)] = sel
            maxn = max(maxn, len(sel))
    E_pad = max(128, ((maxn + 127) // 128) * 128)

    key = (E_pad, KSTAGE)
    if key not in _PROG_CACHE:
        _PROG_CACHE[key] = _build(E_pad, KSTAGE)
    nc = _PROG_CACHE[key]

    in_maps = [_prep_core_inputs(c, xs, eis, eas, params, buckets, E_pad)
               for c in range(NCORES)]
    trace = os.environ.get("KERNEL_TRACE", "0") == "1"
    res = run_bass_kernel_spmd(nc, in_maps, core_ids=list(range(NCORES)), trace=trace)
    LAST_EXEC_NS = res.exec_time_ns
    yv = res.results[0]["y"][0]
    return yv[0:300].copy(), yv[300:600].copy(), yv[600:601].copy()
